# revision 30
# baseline (speedup 1.0000x reference)
"""Trainium2 Bass kernel for nn_Attention_9354438771128.

GQA attention block (Mistral-style): QKV projections + RoPE + block-diagonal
(8 x 1024) full attention + output projection, fp32 reference.

Sharding: data-parallel over the 8 sequence blocks, one block per NeuronCore.
Each core computes its block's full attention independently (no collectives).

Per-core pipeline (all matmuls bf16 with fp32 PSUM accumulation):
  - host pre-work: x^T slices, per-head even/odd column permutation of wq/wk
    (turns interleaved RoPE into a half-rotation), RoPE cos/sin tables in
    transposed layout, wv/wo pre-tiled so every weight DMA is one fully
    contiguous block.
  - k^T/q^T computed per head directly in [head_dim, seq] layout; RoPE applied
    with 4 DVE ops using partition-offset operands (no SBUF->SBUF swap DMA).
  - per head: scores^T = k^T.T @ q^T into a 2-bank [128,1024] PSUM tile
    (LDW shared between the two 512-wide matmuls); exp on ScalarE (fused
    scale, no max subtraction -- scores bounded ~|9|); running softmax
    denominator on DVE; PV accumulates A^T = V^T P^T in PSUM; denominator
    all-reduced+broadcast across partitions by GpSimd partition_all_reduce;
    normalization fused into the PSUM evacuation; A^T staged to DRAM (bf16,
    one contiguous store per head).
  - out = A @ wo streamed per 512-column block, one contiguous DMA per block.
"""

import sys

sys.path.insert(0, "/opt/trn_rl_repo")

import numpy as np
import ml_dtypes

BF = ml_dtypes.bfloat16

B, S, DIM = 8, 1024, 4096
NH, NKV, HD = 32, 8, 128
KC = DIM // 128            # 32 contraction chunks
TT = S // 128              # 8 token tiles per block
SCALE = HD ** -0.5

_CACHE = {}


def _build(repeat=1, phases="BDE", loop=0, pipeline_heads=True,
           early_evac=False, psum_alt=False, nogp=False, bf16_rope=False,
           ops_bufs=6, wo_first=False, dma_split=False, at_split=True):
    import concourse.bass as bass
    import concourse.mybir as mybir
    from concourse import bacc, bass_utils
    from concourse.tile import TileContext
    from bass_rust import ReduceOp

    # let walrus elide back-to-back identical weight loads
    if not getattr(bass_utils.get_walrus_args, "_ldw_opt", False):
        _orig = bass_utils.get_walrus_args

        def _patched(*a, **k):
            return [x.replace("--enable-ldw-opt=false", "--enable-ldw-opt=true")
                    for x in _orig(*a, **k)]

        _patched._ldw_opt = True
        bass_utils.get_walrus_args = _patched

    f32 = mybir.dt.float32
    bf16 = mybir.dt.bfloat16
    Exp = mybir.ActivationFunctionType.Exp
    mult = mybir.AluOpType.mult
    add = mybir.AluOpType.add

    nc = bacc.Bacc("TRN2", num_devices=8)

    xT = nc.dram_tensor("xT", [KC, 128, S], bf16, kind="ExternalInput")
    wq = nc.dram_tensor("wq", [NH, 128, DIM], bf16, kind="ExternalInput")
    wk = nc.dram_tensor("wk", [NKV, 128, DIM], bf16, kind="ExternalInput")
    wv = nc.dram_tensor("wv", [2, 128, KC, 512], bf16, kind="ExternalInput")
    wo = nc.dram_tensor("wo", [8, 128, NH, 512], bf16, kind="ExternalInput")
    cosb = nc.dram_tensor("cosb", [128, S], f32, kind="ExternalInput")
    sinb = nc.dram_tensor("sinb", [128, S], f32, kind="ExternalInput")
    out = nc.dram_tensor("out", [TT, 128, DIM], f32, kind="ExternalOutput")
    at_dram = nc.dram_tensor("at_scratch", [NH, 128, S], bf16, kind="Internal")

    def body(tc):
        with tc.tile_pool(name="const", bufs=1) as cpool:
            cos_t = cpool.tile([128, S], f32)
            sin_t = cpool.tile([128, S], f32)
            nc.sync.dma_start(cos_t, cosb[:])
            nc.sync.dma_start(sin_t, sinb[:])
            if nogp:
                ones_col = cpool.tile([128, 1], bf16)
                ones_row = cpool.tile([1, 128], bf16)
                nc.vector.memset(ones_col, 1.0)
                nc.vector.memset(ones_row, 1.0)
            if bf16_rope:
                cos_tb = cpool.tile([128, S], bf16)
                sin_tb = cpool.tile([128, S], bf16)
                with nc.allow_low_precision(reason="bf16 rope tables"):
                    nc.vector.tensor_copy(cos_tb, cos_t)
                    nc.vector.tensor_copy(sin_tb, sin_t)

            def rope_store(psum_half, dst, sl):
                # dst = psum * cos + halfswap(psum) * sin  (sign folded into
                # the host-built sin table); partition-offset DVE operands
                # replace the SBUF->SBUF partition-swap DMA.
                if bf16_rope:
                    # ACT evacuates PSUM to bf16 so all 4 DVE ops run in the
                    # 2x 16-bit mode (probe: shifts ~2.8us/head DVE -> ACT)
                    raw = rpool.tile([128, 512], bf16, tag="rope_raw")
                    with nc.allow_low_precision(reason="bf16 rope"):
                        nc.scalar.copy(raw, psum_half)
                        t1 = rpool.tile([128, 512], bf16, tag="rope_t1")
                        t2 = rpool.tile([128, 512], bf16, tag="rope_t2")
                        nc.vector.tensor_tensor(t1, raw, cos_tb[:, sl], mult)
                        nc.vector.tensor_tensor(
                            t2[0:64], raw[64:128], sin_tb[0:64, sl], mult)
                        nc.vector.tensor_tensor(
                            t2[64:128], raw[0:64], sin_tb[64:128, sl], mult)
                        nc.vector.tensor_tensor(dst, t1, t2, add)
                    return
                t1 = rpool.tile([128, 512], f32, tag="rope_t1")
                t2 = rpool.tile([128, 512], f32, tag="rope_t2")
                nc.vector.tensor_tensor(t1, psum_half, cos_t[:, sl], mult)
                nc.vector.tensor_tensor(
                    t2[0:64], psum_half[64:128], sin_t[0:64, sl], mult)
                nc.vector.tensor_tensor(
                    t2[64:128], psum_half[0:64], sin_t[64:128, sl], mult)
                nc.vector.tensor_tensor(dst, t1, t2, add)

            with tc.tile_pool(name="xt", bufs=1) as xtpool, \
                 tc.tile_pool(name="wstream", bufs=3) as wpool, \
                 tc.tile_pool(name="rope", bufs=2) as rpool, \
                 tc.tile_pool(name="kv", bufs=1) as kvpool, \
                 tc.tile_pool(name="qkps", bufs=2, space="PSUM") as qkps:
                xt_t = xtpool.tile([128, KC, S], bf16)
                for kc in range(KC):
                    nc.sync.dma_start(xt_t[:, kc], xT[kc])

                kt_t = kvpool.tile([128, NKV, S], bf16)
                v_t = kvpool.tile([128, TT, NKV * HD], bf16)

                # ---------------- Phase B: K^T (roped) and V ----------------
                for g in range(NKV):
                    wk_t = wpool.tile([128, DIM], bf16, tag="wqk")
                    nc.sync.dma_start(wk_t, wk[g])
                    for ch in range(2):
                        sl = slice(ch * 512, (ch + 1) * 512)
                        ps = qkps.tile([128, 512], f32, tag="qk")
                        for kc in range(KC):
                            nc.tensor.matmul(
                                ps, wk_t[:, kc * 128:(kc + 1) * 128],
                                xt_t[:, kc, sl],
                                start=(kc == 0), stop=(kc == KC - 1))
                        rope_store(ps, kt_t[:, g, sl], sl)

                with tc.tile_pool(name="wvstream", bufs=2) as wvpool, \
                     tc.tile_pool(name="vps", bufs=2, space="PSUM") as vps:
                    for vc in range(2):
                        wv_t = wvpool.tile([128, KC, 512], bf16)
                        nc.sync.dma_start(wv_t, wv[vc])
                        for tt in range(TT):
                            ps = vps.tile([128, 512], f32)
                            for kc in range(KC):
                                nc.tensor.matmul(
                                    ps, xt_t[:, kc, tt * 128:(tt + 1) * 128],
                                    wv_t[:, kc],
                                    start=(kc == 0), stop=(kc == KC - 1))
                            nc.vector.tensor_copy(
                                v_t[:, tt, vc * 512:(vc + 1) * 512], ps)

                # ---------------- Phase D: per-head Q + attention ----------------
                if "D" not in phases:
                    nc.gpsimd.dma_start(
                        out[0, :, :S],
                        kt_t.rearrange("p a b -> p (a b)")[:, :S])
                    nc.gpsimd.dma_start(
                        out[1, :, :S],
                        v_t.rearrange("p a b -> p (a b)")[:, :S])
                    return
                with tc.tile_pool(name="qt", bufs=3) as qtpool, \
                     tc.tile_pool(name="expt", bufs=4) as epool, \
                     tc.tile_pool(name="esum", bufs=4) as espool, \
                     tc.tile_pool(name="nrm", bufs=2) as npool, \
                     tc.tile_pool(name="atst", bufs=2) as atpool, \
                     tc.tile_pool(name="sps", bufs=(1 if psum_alt else 2),
                                  space="PSUM") as sps, \
                     tc.tile_pool(name="aps", bufs=(2 if psum_alt else 1),
                                  space="PSUM") as aps:

                    def qproj(h, qt_t):
                        wq_t = wpool.tile([128, DIM], bf16, tag="wqk")
                        nc.sync.dma_start(wq_t, wq[h])
                        for ch in range(2):
                            sl = slice(ch * 512, (ch + 1) * 512)
                            ps = qkps.tile([128, 512], f32, tag="qk")
                            for kc in range(KC):
                                nc.tensor.matmul(
                                    ps, wq_t[:, kc * 128:(kc + 1) * 128],
                                    xt_t[:, kc, sl],
                                    start=(kc == 0), stop=(kc == KC - 1))
                            rope_store(ps, qt_t[:, sl], sl)

                    qts = {}
                    if pipeline_heads:
                        qts[0] = qtpool.tile([128, S], bf16, tag="qt_t", name="qt_t")
                        qproj(0, qts[0])
                    for h in range(NH):
                        g = h // 4
                        if pipeline_heads:
                            if h + 1 < NH:
                                qts[h + 1] = qtpool.tile(
                                    [128, S], bf16, tag="qt_t", name="qt_t")
                                qproj(h + 1, qts[h + 1])
                            qt_t = qts.pop(h)
                        else:
                            qt_t = qtpool.tile([128, S], bf16, tag="qt_t")
                            qproj(h, qt_t)

                        a_ps = aps.tile([128, S], f32)
                        es_prev = None
                        for sk in range(TT):
                            kt_sl = kt_t[:, g, sk * 128:(sk + 1) * 128]
                            s_ps = sps.tile([128, S], f32)
                            nc.tensor.matmul(
                                s_ps[:, 0:512], kt_sl, qt_t[:, 0:512],
                                start=True, stop=True)
                            nc.tensor.matmul(
                                s_ps[:, 512:1024], kt_sl, qt_t[:, 512:1024],
                                start=True, stop=True)
                            e_t = epool.tile([128, S], bf16)
                            nc.scalar.activation(e_t, s_ps, Exp, scale=SCALE)
                            if es_prev is None:
                                es_prev = e_t
                            else:
                                es_new = espool.tile([128, S], bf16, tag="es")
                                nc.vector.tensor_tensor(es_new, es_prev, e_t, add)
                                es_prev = es_new
                            v_sl = v_t[:, sk, g * 128:(g + 1) * 128]
                            nc.tensor.matmul(
                                a_ps[:, 0:512], v_sl, e_t[:, 0:512],
                                start=(sk == 0), stop=(sk == TT - 1))
                            nc.tensor.matmul(
                                a_ps[:, 512:1024], v_sl, e_t[:, 512:1024],
                                start=(sk == 0), stop=(sk == TT - 1))

                        if nogp:
                            # denominator without GpSimd (its SBUF port lock
                            # stalls DVE): PE ones-matmul partition reduction,
                            # DVE reciprocal, PE outer-product broadcast,
                            # ScalarE copy to SBUF for the DVE normalize.
                            n_ps0 = qkps.tile([1, 512], f32, tag="qk",
                                              name="n_ps0")
                            n_ps1 = qkps.tile([1, 512], f32, tag="qk",
                                              name="n_ps1")
                            nc.tensor.matmul(n_ps0, ones_col,
                                             es_prev[:, 0:512],
                                             start=True, stop=True)
                            nc.tensor.matmul(n_ps1, ones_col,
                                             es_prev[:, 512:1024],
                                             start=True, stop=True)
                            rec1_t = npool.tile([1, S], bf16, tag="rec1")
                            with nc.allow_low_precision(
                                    reason="bf16 reciprocal feeds bf16 matmul"):
                                nc.vector.reciprocal(rec1_t[:, 0:512], n_ps0)
                                nc.vector.reciprocal(
                                    rec1_t[:, 512:1024], n_ps1)
                            nb_ps = sps.tile([128, S], f32, tag="s_ps",
                                             name="nb_ps")
                            nc.tensor.matmul(nb_ps[:, 0:512], ones_row,
                                             rec1_t[:, 0:512],
                                             start=True, stop=True)
                            nc.tensor.matmul(nb_ps[:, 512:1024], ones_row,
                                             rec1_t[:, 512:1024],
                                             start=True, stop=True)
                            rec_t = npool.tile([128, S], f32, tag="nb")
                            nc.scalar.copy(rec_t, nb_ps)
                        else:
                            nb_t = npool.tile([128, S], f32, tag="nb")
                            nc.gpsimd.partition_all_reduce(
                                nb_t, es_prev, 128, ReduceOp.add)
                            rec_t = npool.tile([128, S], f32, tag="rec")
                            nc.vector.reciprocal(rec_t, nb_t)
                        at_t = atpool.tile([128, S], bf16)
                        if early_evac:
                            # free the PSUM bank before the gpsimd/recip chain
                            # completes so the next head's PV is not gated on it
                            ar_t = npool.tile([128, S], f32, tag="ar")
                            nc.vector.tensor_copy(ar_t, a_ps)
                            nc.vector.tensor_tensor(at_t, ar_t, rec_t, mult)
                        else:
                            nc.vector.tensor_tensor(at_t, a_ps, rec_t, mult)
                        if dma_split:
                            nc.scalar.dma_start(at_dram[h], at_t)
                        else:
                            nc.sync.dma_start(at_dram[h], at_t)

        # ---------------- Phase E: out = A @ wo ----------------
        if "E" not in phases:
            return
        with tc.tile_pool(name="atrd", bufs=(NH if at_split else 1)) as atrd, \
             tc.tile_pool(name="wo", bufs=3) as wopool, \
             tc.tile_pool(name="ost", bufs=6) as opool, \
             tc.tile_pool(name="ops", bufs=ops_bufs, space="PSUM") as ops:
            wo_pre = []
            if wo_first:
                wo_t0 = wopool.tile([128, NH, 512], bf16, tag="wo_t")
                nc.sync.dma_start(wo_t0, wo[0])
                wo_pre.append(wo_t0)
            if at_split:
                # one tile per head: E matmuls gate on individual A^T loads
                at_hs = []
                for h in range(NH):
                    ath = atrd.tile([128, S], bf16, tag="at_h", name="at_h")
                    nc.sync.dma_start(ath, at_dram[h])
                    at_hs.append(ath)
                at_sl = lambda h, tt: at_hs[h][:, tt * 128:(tt + 1) * 128]
            else:
                at_all = atrd.tile([128, NH, S], bf16)
                for h in range(NH):
                    if dma_split:
                        nc.scalar.dma_start(at_all[:, h], at_dram[h])
                    else:
                        nc.sync.dma_start(at_all[:, h], at_dram[h])
                at_sl = lambda h, tt: at_all[:, h, tt * 128:(tt + 1) * 128]
            for nch in range(8):
                nsl = slice(nch * 512, (nch + 1) * 512)
                if nch < len(wo_pre):
                    wo_t = wo_pre[nch]
                else:
                    wo_t = wopool.tile([128, NH, 512], bf16, tag="wo_t")
                    nc.sync.dma_start(wo_t, wo[nch])
                for tt in range(TT):
                    o_ps = ops.tile([128, 512], f32)
                    for h in range(NH):
                        nc.tensor.matmul(
                            o_ps, at_sl(h, tt), wo_t[:, h],
                            start=(h == 0), stop=(h == NH - 1))
                    o_t = opool.tile([128, 512], f32)
                    nc.vector.tensor_copy(o_t, o_ps)
                    nc.sync.dma_start(out[tt, :, nsl], o_t)

    with TileContext(nc) as tc:
        if loop > 1:
            with tc.For_i(0, loop, 1):
                body(tc)
        else:
            for _rep in range(repeat):
                body(tc)

    nc.compile()
    return nc


def _prep_shared(wq, wk, wv, wo):
    idx = np.arange(128)
    ph = np.concatenate([idx[0::2], idx[1::2]])
    permq = (np.arange(NH)[:, None] * HD + ph[None, :]).reshape(-1)
    permk = (np.arange(NKV)[:, None] * HD + ph[None, :]).reshape(-1)
    wq_r = np.ascontiguousarray(
        wq[:, permq].reshape(KC, 128, NH, HD).transpose(2, 1, 0, 3)
    ).reshape(NH, 128, DIM).astype(BF)
    wk_r = np.ascontiguousarray(
        wk[:, permk].reshape(KC, 128, NKV, HD).transpose(2, 1, 0, 3)
    ).reshape(NKV, 128, DIM).astype(BF)
    wv_r = np.ascontiguousarray(
        wv.reshape(KC, 128, 2, 512).transpose(2, 1, 0, 3)).astype(BF)
    wo_r = np.ascontiguousarray(
        wo.reshape(NH, 128, 8, 512).transpose(2, 1, 0, 3)).astype(BF)
    return wq_r, wk_r, wv_r, wo_r


def make_in_maps(x, freqs_cos, freqs_sin, wq, wk, wv, wo):
    wq_r, wk_r, wv_r, wo_r = _prep_shared(
        np.asarray(wq, np.float32), np.asarray(wk, np.float32),
        np.asarray(wv, np.float32), np.asarray(wo, np.float32))

    x = np.asarray(x, np.float32)
    fc = np.asarray(freqs_cos, np.float32)
    fs = np.asarray(freqs_sin, np.float32)

    in_maps = []
    for b in range(B):
        xb = x[b * S:(b + 1) * S]                       # [S, DIM]
        xT_b = np.ascontiguousarray(xb.T).astype(BF).reshape(KC, 128, S)
        c = np.ascontiguousarray(fc[b * S:(b + 1) * S].T.astype(np.float32))
        s = np.ascontiguousarray(fs[b * S:(b + 1) * S].T.astype(np.float32))
        cosb = np.concatenate([c, c], axis=0)           # [128, S]
        sinb = np.concatenate([-s, s], axis=0)
        in_maps.append({
            "xT": xT_b, "wq": wq_r, "wk": wk_r, "wv": wv_r, "wo": wo_r,
            "cosb": np.ascontiguousarray(cosb),
            "sinb": np.ascontiguousarray(sinb),
        })
    return in_maps


def kernel(x, freqs_cos, freqs_sin, wq, wk, wv, wo):
    from concourse.bass_utils import run_bass_kernel_spmd

    if "nc" not in _CACHE:
        _CACHE["nc"] = _build()
    nc = _CACHE["nc"]

    in_maps = make_in_maps(x, freqs_cos, freqs_sin, wq, wk, wv, wo)

    res = run_bass_kernel_spmd(nc, in_maps, core_ids=list(range(B)))
    _CACHE["last_results"] = res
    outs = [r["out"].reshape(S, DIM) for r in res.results]
    return np.concatenate(outs, axis=0)
